# revision 26
# baseline (speedup 1.0000x reference)
"""Trainium2 Bass kernel for nn_DecoderCacheLayer (LTM/WM read -> causal dilated
conv stack -> WM/LTM write).

Sharding: 8 cores = (batch b in 0..4) x (sequence half in 0..2). Each core runs
the full fused pipeline on its 2048 tokens (odd halves recompute a 16-token
halo; even halves use the zero-padding semantics via a mask). Cache/WM write
phases produce per-core partial sums ([K,DC+1]/[KW,DC+1]); the host reduces the
two halves of each batch and applies the tiny tanh/blend finalization.

On-device layout is "transposed": features on partitions, tokens on the free
dim. All matmuls run in fp32r (fp32 with 11-bit mantissa, full PE rate).
"""

import numpy as np
from contextlib import ExitStack
from dataclasses import dataclass

import concourse.bass as bass
import concourse.tile as tile
from concourse import bacc, mybir
from concourse.bass_utils import run_bass_kernel_spmd
from concourse.masks import make_identity

F32 = mybir.dt.float32
F32R = mybir.dt.float32r
BF16 = mybir.dt.bfloat16
I32 = mybir.dt.int32
AF = mybir.ActivationFunctionType
ALU = mybir.AluOpType


@dataclass(frozen=True)
class Cfg:
    D: int = 1024          # model dim
    DC: int = 256          # cache dim
    K: int = 64            # LTM slots
    KW: int = 8            # WM slots
    TAPS: int = 5          # conv kernel
    DILS: tuple = (1, 2)   # conv dilations
    S_CORE: int = 2048     # real tokens per core
    HALO: int = 16         # halo tokens recomputed at core start
    SEG: int = 512         # segment length (conv/LN phase width)
    W_PH: int = 256        # subtile width for phase12 / postLN / write phases
    EPS: float = 1e-5
    SC: float = 1.0 / 16.0  # 1/sqrt(DC)

    @property
    def ND(self):
        return self.D // 128

    @property
    def NDC(self):
        return self.DC // 128

    @property
    def HB(self):
        return max((self.TAPS - 1) * d for d in self.DILS)  # lnh lead halo cols

    @property
    def NSEG(self):
        return self.S_CORE // self.SEG


FULL = Cfg()
N_CORES = 8


def round_f32r(x: np.ndarray) -> np.ndarray:
    u = np.ascontiguousarray(x, dtype=np.float32).view(np.uint32)
    r = u + 0x7FF + ((u >> 12) & 1)
    r &= np.uint32(0xFFFFF000)
    return r.view(np.float32)


# ---------------------------------------------------------------------------
# device program
# ---------------------------------------------------------------------------

class K:
    """Builder for the per-core SPMD program."""

    def __init__(self, cfg: Cfg):
        self.cfg = cfg
        nc = bacc.Bacc("TRN2", target_bir_lowering=False, debug=False)
        self.nc = nc
        c = cfg
        dt_in = {}

        def din(name, shape, dt=F32R):
            dt_in[name] = nc.dram_tensor(name, shape, dt, kind="ExternalInput").ap()

        din("x_sh", [c.HALO + c.S_CORE, c.D])  # F32R: host pre-rounds x
        din("cache_n", [c.K, c.DC])
        din("cache_t", [c.DC, c.K])
        din("cont_n", [c.KW, c.DC])
        din("cont_t", [c.DC, c.KW])
        din("valid_b", [128, c.KW], F32)
        din("mask", [128, c.HALO], F32)
        # packed per-feature vectors: s0,b0,s1,b1,post_s,post_b,cb0,cb1
        din("lnp", [128, 8 * c.ND], F32)
        din("wq_ltm", [c.D, c.DC])
        din("wo_ltm", [c.DC, c.D])
        din("wgr_ltm", [c.D, 2])
        din("wq_wm", [c.D, c.DC])
        din("wo_wm", [c.DC, c.D])
        din("wgr_wm", [c.D, 2])
        din("wv_wm", [c.D, c.DC])
        din("ws_wm", [c.D, c.KW])
        din("wgw_pair", [c.D, 2])
        din("wv_ltm", [c.D, c.DC])
        din("cw0", [c.ND, 128, c.TAPS * c.ND * 128], BF16)
        din("cw1", [c.ND, 128, c.TAPS * c.ND * 128], BF16)
        self.i = dt_in
        self.o = {
            "out_sh": nc.dram_tensor("out_sh", [c.S_CORE, c.D], F32,
                                     kind="ExternalOutput").ap(),
            "wm_stats": nc.dram_tensor("wm_stats", [c.KW, c.DC + 1], F32,
                                       kind="ExternalOutput").ap(),
            "ltm_stats": nc.dram_tensor("ltm_stats", [c.K, c.DC + 1], F32,
                                        kind="ExternalOutput").ap(),
        }

    # -- small helpers ------------------------------------------------------

    def mm(self, out, lhsT, rhs, start, stop):
        self.nc.tensor.matmul(out, lhsT, rhs, start=start, stop=stop)

    def build(self):
        cfg = self.cfg
        nc = self.nc
        with tile.TileContext(nc) as tc:
            with ExitStack() as ctx:
                self.tc = tc
                p = lambda name, bufs, **kw: ctx.enter_context(
                    tc.tile_pool(name=name, bufs=bufs, **kw))
                # SBUF pools
                self.pw = p("weights", 1)       # persistent weights / consts
                self.pcw = p("convw", 2)        # streamed conv weights
                self.pbuf = p("bufs", 2)        # h / lnh segment buffers
                self.pxt = p("xt", 2)           # xT staging (also x1/x2 in place)
                self.pot = p("outT", 2)         # postLN ^T staging
                self.pnat = p("nat", 2)         # natural-layout staging tiles
                self.pmid = p("mid", 2)         # qt/rt/vt phase intermediates
                self.ptmp = p("tmp", 2)         # elementwise temporaries
                self.psml = p("small", 2)       # small per-block vectors
                self.pstat = p("stat", 1)       # [1,W] stat vectors
                self.pst = p("stash", 2)        # lnh halo stashes between segments
                self.pacc = p("acc", 1)         # write-phase accumulators
                # PSUM pools (8 banks total)
                self.ps_mm = p("ps_mm", 2, space="PSUM")   # big matmul outputs
                self.ps_st = p("ps_st", 1, space="PSUM")   # [1,2W] LN sums
                self.ps_tr = p("ps_tr", 3, space="PSUM")   # transposes / scores
                self.ps_acc = p("ps_acc", 1, space="PSUM")  # [K,DC+1] write sums
                self._build_body()
        nc.compile()
        return nc

    # -- static tiles -------------------------------------------------------

    def _consts(self):
        nc, c = self.nc, self.cfg
        idf = self.ptmp.tile([128, 128], F32, tag="g")
        make_identity(nc, idf[:])
        self.ident = self.pw.tile([128, 128], F32R, tag="ident")
        nc.vector.tensor_copy(self.ident[:], idf[:])
        onf = self.ptmp.tile([128, 1], F32, tag="tm")
        nc.vector.memset(onf[:], 1.0)
        self.ones_col = self.pw.tile([128, 1], F32R, tag="ones_col")
        nc.vector.tensor_copy(self.ones_col[:], onf[:])
        onr = self.ptmp.tile([1, 128], F32, tag="tm")
        nc.vector.memset(onr[:], 1.0)
        self.ones_row = self.pw.tile([1, 128], F32R, tag="ones_row")
        nc.vector.tensor_copy(self.ones_row[:], onr[:])
        self.eps_t = self.pw.tile([1, 1], F32, tag="eps")
        nc.vector.memset(self.eps_t[:], c.EPS)
        a2f = self.ptmp.tile([128, 2], F32, tag="tm")
        nc.vector.memset(a2f[:, 0:1], 1.0)
        nc.vector.memset(a2f[:, 1:2], 0.0)
        self.aug2 = self.pw.tile([128, 2], F32R, tag="aug2")
        nc.vector.tensor_copy(self.aug2[:], a2f[:])
        # rsqrt Newton constants: magic seed + shift amount, int32 rows
        self.magic_i = self.pw.tile([1, 256], I32, tag="magic")
        nc.vector.memset(self.magic_i[:], 0x5F3759DF)
        self.one_i = self.pw.tile([1, 256], I32, tag="onei")
        nc.vector.memset(self.one_i[:], 1)

    def _load_weights(self):
        nc, c = self.nc, self.cfg
        ND, NDC = c.ND, c.NDC

        # [D, X] weights -> SBUF [128, ND*X] (chunk-major free layout)
        def ld_dx(name, X):
            t = self.pw.tile([128, ND * X], F32R, tag=name)
            nc.sync.dma_start(
                t[:].rearrange("p (n x) -> p n x", n=ND),
                self.i[name].rearrange("(n p) x -> p n x", p=128))
            return t

        # [DC, X] weights -> SBUF [128, NDC*X]
        def ld_cx(name, X):
            t = self.pw.tile([128, NDC * X], F32R, tag=name)
            nc.sync.dma_start(
                t[:].rearrange("p (n x) -> p n x", n=NDC),
                self.i[name].rearrange("(n p) x -> p n x", p=128))
            return t

        self.wq_ltm = ld_dx("wq_ltm", c.DC)
        self.wq_wm = ld_dx("wq_wm", c.DC)
        self.wv_wm = ld_dx("wv_wm", c.DC)
        self.wv_ltm = ld_dx("wv_ltm", c.DC)
        self.ws_wm = ld_dx("ws_wm", c.KW)
        self.wgr_ltm = ld_dx("wgr_ltm", 2)
        self.wgr_wm = ld_dx("wgr_wm", 2)
        self.wgw_pair = ld_dx("wgw_pair", 2)

        self.wo_ltm = ld_cx("wo_ltm", c.D)
        self.wo_wm = ld_cx("wo_wm", c.D)

        self.cache_n = self.pw.tile([c.K, c.DC], F32R, tag="cache_n")
        nc.sync.dma_start(self.cache_n[:], self.i["cache_n"])
        self.cache_t = self.pw.tile([128, NDC * c.K], F32R, tag="cache_t")
        nc.sync.dma_start(
            self.cache_t[:].rearrange("p (n x) -> p n x", n=NDC),
            self.i["cache_t"].rearrange("(n p) x -> p n x", p=128))
        self.cont_n = self.pw.tile([c.KW, c.DC], F32R, tag="cont_n")
        nc.sync.dma_start(self.cont_n[:], self.i["cont_n"])
        self.cont_t = self.pw.tile([128, NDC * c.KW], F32R, tag="cont_t")
        nc.sync.dma_start(
            self.cont_t[:].rearrange("p (n x) -> p n x", n=NDC),
            self.i["cont_t"].rearrange("(n p) x -> p n x", p=128))
        self.valid_b = self.pw.tile([128, c.KW], F32, tag="valid_b")
        nc.sync.dma_start(self.valid_b[:], self.i["valid_b"])
        self.mask = self.pw.tile([128, c.HALO], F32, tag="mask")
        nc.sync.dma_start(self.mask[:], self.i["mask"])
        self.lnp = self.pw.tile([128, 8 * ND], F32, tag="lnp")
        nc.sync.dma_start(self.lnp[:], self.i["lnp"])

    def lnp_col(self, which, dk):
        # order: s0,b0,s1,b1,post_s,post_b,cb0,cb1
        idx = {"s0": 0, "b0": 1, "s1": 2, "b1": 3,
               "ps": 4, "pb": 5, "cb0": 6, "cb1": 7}[which]
        return self.lnp[:, idx * self.cfg.ND + dk: idx * self.cfg.ND + dk + 1]

    # -- building blocks ----------------------------------------------------

    def load_xT(self, row0, W):
        """DMA x rows [row0, row0+W) and transpose into an f32r ^T tile
        [128, ND*W] (feature chunk-major)."""
        nc, c = self.nc, self.cfg
        xt = self.pxt.tile([128, c.ND * W], F32R, tag="xt")
        for blk0 in range(0, W, 128):
            bw = min(128, W - blk0)
            nat = self.pnat.tile([128, c.D], F32R, tag="nat")
            nc.sync.dma_start(nat[:bw, :], self.i["x_sh"][row0 + blk0:
                                                          row0 + blk0 + bw, :])
            for dk in range(c.ND):
                tp = self.ps_tr.tile([128, 128], F32R, tag="tr")
                nc.tensor.transpose(tp[:128, :bw],
                                    nat[:bw, dk * 128:(dk + 1) * 128],
                                    self.ident[:bw, :bw])
                nc.scalar.copy(xt[:, dk * W + blk0: dk * W + blk0 + bw],
                               tp[:128, :bw])
        return xt

    def read_phase(self, xt, W, kind, out_at=None):
        """LTM/WM read: xt (in/out, f32r ^T [128, ND*W]) updated in place:
        x <- x + sigmoid(x@wgr) * ((softmax(x@Wq @ memT)*[valid]) @ mem @ Wo)."""
        nc, c = self.nc, self.cfg
        ND, NDC = c.ND, c.NDC
        if kind == "ltm":
            wq, wo, wgr = self.wq_ltm, self.wo_ltm, self.wgr_ltm
            mem_n, mem_t, nk = self.cache_n, self.cache_t, c.K
        else:
            wq, wo, wgr = self.wq_wm, self.wo_wm, self.wgr_wm
            mem_n, mem_t, nk = self.cont_n, self.cont_t, c.KW

        # q^T = (x @ Wq)^T   [DC, W]
        qt = self.pmid.tile([128, NDC * W], F32R, tag="qt")
        for dcc in range(NDC):
            ps = self.ps_mm.tile([128, W], F32, tag="mm")
            for dk in range(ND):
                self.mm(ps[:], wq[:, dk * c.DC + dcc * 128: dk * c.DC + dcc * 128 + 128],
                        xt[:, dk * W:(dk + 1) * W], dk == 0, dk == ND - 1)
            nc.scalar.copy(qt[:, dcc * W:(dcc + 1) * W], ps[:])
        # attention per 128-token block; the sigmoid read-gate is folded into
        # the attention weights (linear in the read), so the residual is a
        # plain add.
        at = self.ptmp.tile([nk, W], F32R, tag="at")
        for blk0 in range(0, W, 128):
            bw = min(128, W - blk0)
            # natural gate logits [bw, 2] (col 0 = wgr, col 1 = zero pad)
            psg = self.ps_tr.tile([128, 2], F32, tag="tr")
            for dk in range(ND):
                self.mm(psg[:bw, :], xt[:, dk * W + blk0: dk * W + blk0 + bw],
                        wgr[:, dk * 2:(dk + 1) * 2], dk == 0, dk == ND - 1)
            gn = self.psml.tile([128, 2], F32, tag="gn")
            nc.scalar.activation(gn[:bw, :], psg[:bw, :], AF.Tanh, scale=0.5)
            pss = self.ps_tr.tile([128, nk], F32, tag="tr")
            for dcc in range(NDC):
                self.mm(pss[:bw, :], qt[:, dcc * W + blk0: dcc * W + blk0 + bw],
                        mem_t[:, dcc * nk:(dcc + 1) * nk], dcc == 0, dcc == NDC - 1)
            ex = self.psml.tile([128, nk], F32, tag="ex")
            nc.scalar.activation(ex[:bw, :], pss[:bw, :], AF.Exp, scale=c.SC)
            rs = self.psml.tile([128, 1], F32, tag="rs")
            nc.vector.tensor_reduce(rs[:bw, :], ex[:bw, :], mybir.AxisListType.X,
                                    ALU.add)
            rc = self.psml.tile([128, 1], F32, tag="rc")
            nc.vector.reciprocal(rc[:bw, :], rs[:bw, :])
            # rc *= sigmoid(gate) = 0.5*tanh(0.5 g) + 0.5
            sig = self.psml.tile([128, 1], F32, tag="sig")
            nc.vector.tensor_scalar(sig[:bw, :], gn[:bw, 0:1], 0.5, 0.5,
                                    ALU.mult, ALU.add)
            nc.vector.tensor_mul(rc[:bw, :], rc[:bw, :], sig[:bw, :])
            an = self.psml.tile([128, nk], F32R, tag="an")
            if kind == "wm":
                nc.vector.scalar_tensor_tensor(an[:bw, :], ex[:bw, :], rc[:bw, :],
                                               self.valid_b[:bw, :],
                                               ALU.mult, ALU.mult)
            else:
                nc.vector.tensor_scalar_mul(an[:bw, :], ex[:bw, :], rc[:bw, :])
            ptt = self.ps_tr.tile([128, 128], F32R, tag="tr")
            nc.tensor.transpose(ptt[:nk, :bw], an[:bw, :nk], self.ident[:bw, :bw])
            nc.vector.tensor_copy(at[:, blk0:blk0 + bw], ptt[:nk, :bw])
        # read^T [DC, W] then out-proj + residual, all per chunk
        rt = self.pmid.tile([128, NDC * W], F32R, tag="rt")
        for dcc in range(NDC):
            ps = self.ps_mm.tile([128, W], F32, tag="mm")
            self.mm(ps[:], mem_n[:, dcc * 128:(dcc + 1) * 128], at[:], True, True)
            nc.scalar.copy(rt[:, dcc * W:(dcc + 1) * W], ps[:])
        for mc in range(ND):
            ps = self.ps_mm.tile([128, W], F32, tag="mm")
            for dcc in range(NDC):
                self.mm(ps[:], wo[:, dcc * c.D + mc * 128: dcc * c.D + mc * 128 + 128],
                        rt[:, dcc * W:(dcc + 1) * W], dcc == 0, dcc == NDC - 1)
            dst = out_at(mc) if out_at is not None else xt[:, mc * W:(mc + 1) * W]
            nc.vector.tensor_add(dst,
                                 xt[:, mc * W:(mc + 1) * W].bitcast(F32), ps[:])

    def layer_norm(self, in_at, out_at, W, s_col, b_col):
        """Partition-axis LN in ^T layout. in_at/out_at: fn(dk)->AP [128, W].
        s_col/b_col: fn(dk)->AP [128,1]."""
        nc, c = self.nc, self.cfg
        ND = c.ND
        psst = self.ps_st.tile([1, 2 * W], F32, tag="st")
        pssum = psst[:, 0:W]
        pssq = psst[:, W:2 * W]
        for dk in range(ND):
            self.mm(pssum, self.ones_col[:], in_at(dk), dk == 0, dk == ND - 1)
        for dk in range(ND):
            a = in_at(dk)
            sq = self.ptmp.tile([128, W], F32R, tag="lntmp")
            nc.vector.tensor_mul(sq[:], a.bitcast(F32), a.bitcast(F32))
            self.mm(pssq, self.ones_col[:], sq[:], dk == 0, dk == ND - 1)
        inv_d = 1.0 / c.D
        meanf = self.pstat.tile([1, W], F32, tag="meanf")
        nc.scalar.mul(meanf[:], pssum, inv_d)
        msq = self.pstat.tile([1, W], F32, tag="msq")
        nc.scalar.square(msq[:], meanf[:])
        var = self.pstat.tile([1, W], F32, tag="var")
        nc.vector.scalar_tensor_tensor(var[:], pssq, inv_d, msq[:],
                                       ALU.mult, ALU.subtract)
        # rstd = exp(-0.5 * ln(var + eps))
        lnv = self.pstat.tile([1, W], F32, tag="lnv")
        nc.scalar.activation(lnv[:], var[:], AF.Ln, bias=self.eps_t[:])
        rstdf = self.pstat.tile([1, W], F32, tag="rstdf")
        nc.scalar.activation(rstdf[:], lnv[:], AF.Exp, scale=-0.5)
        rstd = self.pstat.tile([1, W], F32R, tag="rstd")
        nc.vector.tensor_copy(rstd[:], rstdf[:])
        mean = self.pstat.tile([1, W], F32R, tag="mean")
        nc.vector.tensor_copy(mean[:], meanf[:])
        psmb = self.ps_mm.tile([128, W], F32, tag="mm")
        self.mm(psmb[:], self.ones_row[:], mean[:], True, True)
        psrb = self.ps_mm.tile([128, W], F32, tag="mm")
        self.mm(psrb[:], self.ones_row[:], rstd[:], True, True)
        mb = self.pstat.tile([128, W], F32, tag="meanb")
        nc.scalar.copy(mb[:], psmb[:])
        rb = self.pstat.tile([128, W], F32, tag="rstdb")
        nc.scalar.copy(rb[:], psrb[:])
        for dk in range(ND):
            t1 = self.ptmp.tile([128, W], F32, tag="lntmp")
            nc.vector.tensor_sub(t1[:], in_at(dk).bitcast(F32), mb[:])
            dst = out_at(dk)
            dstr = dst.bitcast(F32) if dst.dtype == F32R else dst
            nc.vector.scalar_tensor_tensor(dst, t1[:], s_col(dk), rb[:],
                                           ALU.mult, ALU.mult)
            nc.vector.tensor_scalar_add(dst, dstr, b_col(dk))

    def conv_layer(self, layer, lnh_at, h_at, W):
        """One causal dilated conv + gelu + residual (bf16 weights/inputs).
        lnh_at(dk, off, ww) -> AP window; h_at(mc, w0, ww) -> dest cols."""
        nc, c = self.nc, self.cfg
        dil = c.DILS[layer]
        cw = self.i[f"cw{layer}"]
        bias = lambda mc: self.lnp_col(f"cb{layer}", mc)
        for mc in range(c.ND):
            wt = self.pcw.tile([128, c.TAPS * c.ND * 128], BF16, tag="cw")
            nc.sync.dma_start(wt[:], cw[mc])
            for w0 in range(0, W, 512):
                ww = min(512, W - w0)
                ps = self.ps_mm.tile([128, ww], F32, tag="mm")
                n = 0
                for tap in range(c.TAPS):
                    for dk in range(c.ND):
                        self.mm(ps[:],
                                wt[:, (tap * c.ND + dk) * 128:(tap * c.ND + dk + 1) * 128],
                                lnh_at(dk, w0 - tap * dil, ww),
                                n == 0, n == c.TAPS * c.ND - 1)
                        n += 1
                g = self.ptmp.tile([128, ww], F32, tag="g")
                nc.scalar.activation(g[:], ps[:], AF.Gelu_apprx_tanh, bias=bias(mc))
                dst = h_at(mc, w0, ww)
                nc.vector.tensor_add(dst, dst.bitcast(F32), g[:])

    def write_phase(self, ot, W, acc_wm, acc_ltm):
        """WM/LTM write partial sums from out^T tile ot [128, ND*W] (f32r)."""
        nc, c = self.nc, self.cfg
        ND, NDC = c.ND, c.NDC

        # v^T projections [DC, W] for wm and ltm
        def vproj(wv, tag):
            vt = self.pmid.tile([128, NDC * W], F32R, tag=tag)
            for dcc in range(NDC):
                ps = self.ps_mm.tile([128, W], F32, tag="mm")
                for dk in range(ND):
                    self.mm(ps[:],
                            wv[:, dk * c.DC + dcc * 128: dk * c.DC + dcc * 128 + 128],
                            ot[:, dk * W:(dk + 1) * W], dk == 0, dk == ND - 1)
                nc.scalar.copy(vt[:, dcc * W:(dcc + 1) * W], ps[:])
            return vt

        vwt = vproj(self.wv_wm, "vwt")
        vlt = vproj(self.wv_ltm, "vlt")

        ps_nw = self.ps_acc.tile([c.KW, c.DC + 2], F32, tag="accw")
        ps_nl = self.ps_acc.tile([c.K, c.DC + 2], F32, tag="accl")
        nblk = (W + 127) // 128
        for bi in range(nblk):
            blk0 = bi * 128
            bw = min(128, W - blk0)
            # natural gates [bw, 2] = sigmoid(out @ [wgw_wm | wgw_ltm])
            psg = self.ps_tr.tile([128, 2], F32, tag="tr")
            for dk in range(ND):
                self.mm(psg[:bw, :], ot[:, dk * W + blk0: dk * W + blk0 + bw],
                        self.wgw_pair[:, dk * 2:(dk + 1) * 2], dk == 0, dk == ND - 1)
            gnat = self.psml.tile([128, 2], F32, tag="gnat")
            nc.scalar.activation(gnat[:bw, :], psg[:bw, :], AF.Tanh, scale=0.5)
            nc.vector.tensor_scalar(gnat[:bw, :], gnat[:bw, :], 0.5, 0.5,
                                    ALU.mult, ALU.add)
            # ---- WM: softmax(out@Ws) * gate, natural layout
            psw = self.ps_tr.tile([128, c.KW], F32, tag="tr")
            for dk in range(ND):
                self.mm(psw[:bw, :], ot[:, dk * W + blk0: dk * W + blk0 + bw],
                        self.ws_wm[:, dk * c.KW:(dk + 1) * c.KW],
                        dk == 0, dk == ND - 1)
            exw = self.psml.tile([128, c.KW], F32, tag="ex")
            nc.scalar.activation(exw[:bw, :], psw[:bw, :], AF.Exp)
            rsw = self.psml.tile([128, 1], F32, tag="rs")
            nc.vector.tensor_reduce(rsw[:bw, :], exw[:bw, :], mybir.AxisListType.X,
                                    ALU.add)
            rgw = self.psml.tile([128, 1], F32, tag="rg")
            nc.vector.reciprocal(rgw[:bw, :], rsw[:bw, :])
            nc.vector.tensor_mul(rgw[:bw, :], rgw[:bw, :], gnat[:bw, 0:1])
            wv = self.psml.tile([128, c.KW], F32R, tag="wv")
            nc.vector.tensor_scalar_mul(wv[:bw, :], exw[:bw, :], rgw[:bw, :])
            # augmented natural v [bw, DC+2]
            vwa = self.ptmp.tile([128, c.DC + 2], F32R, tag="vaug")
            for dcc in range(NDC):
                pvt = self.ps_tr.tile([128, 128], F32R, tag="tr")
                nc.tensor.transpose(pvt[:bw, :128],
                                    vwt[:, dcc * W + blk0: dcc * W + blk0 + bw],
                                    self.ident[:128, :128])
                nc.vector.tensor_copy(vwa[:bw, dcc * 128:(dcc + 1) * 128],
                                      pvt[:bw, :128])
            nc.vector.tensor_copy(vwa[:bw, c.DC:c.DC + 2],
                                  self.aug2[:bw, :])
            self.mm(ps_nw[:], wv[:bw, :], vwa[:bw, :], bi == 0, bi == nblk - 1)
            # ---- LTM: softmax(v@cacheT * sc) * gate
            psl = self.ps_tr.tile([128, c.K], F32, tag="tr")
            for dcc in range(NDC):
                self.mm(psl[:bw, :], vlt[:, dcc * W + blk0: dcc * W + blk0 + bw],
                        self.cache_t[:, dcc * c.K:(dcc + 1) * c.K],
                        dcc == 0, dcc == NDC - 1)
            exl = self.psml.tile([128, c.K], F32, tag="exl")
            nc.scalar.activation(exl[:bw, :], psl[:bw, :], AF.Exp, scale=c.SC)
            rsl = self.psml.tile([128, 1], F32, tag="rs")
            nc.vector.tensor_reduce(rsl[:bw, :], exl[:bw, :], mybir.AxisListType.X,
                                    ALU.add)
            rgl = self.psml.tile([128, 1], F32, tag="rg")
            nc.vector.reciprocal(rgl[:bw, :], rsl[:bw, :])
            nc.vector.tensor_mul(rgl[:bw, :], rgl[:bw, :], gnat[:bw, 1:2])
            wl = self.psml.tile([128, c.K], F32R, tag="wl")
            nc.vector.tensor_scalar_mul(wl[:bw, :], exl[:bw, :], rgl[:bw, :])
            vla = self.ptmp.tile([128, c.DC + 2], F32R, tag="vaug")
            for dcc in range(NDC):
                pvt = self.ps_tr.tile([128, 128], F32R, tag="tr")
                nc.tensor.transpose(pvt[:bw, :128],
                                    vlt[:, dcc * W + blk0: dcc * W + blk0 + bw],
                                    self.ident[:128, :128])
                nc.vector.tensor_copy(vla[:bw, dcc * 128:(dcc + 1) * 128],
                                      pvt[:bw, :128])
            nc.vector.tensor_copy(vla[:bw, c.DC:c.DC + 2], self.aug2[:bw, :])
            self.mm(ps_nl[:], wl[:bw, :], vla[:bw, :], bi == 0, bi == nblk - 1)
        nc.vector.tensor_add(acc_wm[:], acc_wm[:], ps_nw[:])
        nc.vector.tensor_add(acc_ltm[:], acc_ltm[:], ps_nl[:])

    # -- main body ----------------------------------------------------------

    def _build_body(self):
        nc, c = self.nc, self.cfg
        ND = c.ND
        self._consts()
        self._load_weights()

        acc_wm = self.pacc.tile([c.KW, c.DC + 2], F32, tag="acc_wm")
        nc.vector.memset(acc_wm[:], 0.0)
        acc_ltm = self.pacc.tile([c.K, c.DC + 2], F32, tag="acc_ltm")
        nc.vector.memset(acc_ltm[:], 0.0)

        # ---------------- prologue: halo tokens [0, HALO) -------------------
        H = c.HALO
        W0h = c.DILS[1] * (c.TAPS - 1)   # h1 halo width (8)
        xth = self.load_xT(0, H)
        self.read_phase(xth, H, "ltm")
        self.read_phase(xth, H, "wm")
        # xth now holds x2 (h0) on halo tokens; ln0 -> masked lnh0_halo
        ln0h = self.pst.tile([128, ND * H], BF16, tag="ln0h")
        self.layer_norm(lambda dk: xth[:, dk * H:(dk + 1) * H],
                        lambda dk: ln0h[:, dk * H:(dk + 1) * H], H,
                        lambda dk: self.lnp_col("s0", dk),
                        lambda dk: self.lnp_col("b0", dk))
        for dk in range(ND):
            nc.vector.tensor_mul(ln0h[:, dk * H:(dk + 1) * H],
                                 ln0h[:, dk * H:(dk + 1) * H],
                                 self.mask[:])
        # mini conv0 on halo positions [-W0h, 0)
        h1h = self.pst.tile([128, ND * W0h], F32R, tag="h1h")
        dil0 = c.DILS[0]
        for mc in range(ND):
            wt = self.pcw.tile([128, c.TAPS * c.ND * 128], BF16, tag="cw")
            nc.sync.dma_start(wt[:], self.i["cw0"][mc])
            ps = self.ps_mm.tile([128, W0h], F32, tag="mm")
            n = 0
            for tap in range(c.TAPS):
                for dk in range(ND):
                    off = dk * H + (H - W0h) - tap * dil0
                    self.mm(ps[:], wt[:, (tap * ND + dk) * 128:(tap * ND + dk + 1) * 128],
                            ln0h[:, off: off + W0h], n == 0, n == c.TAPS * ND - 1)
                    n += 1
            g = self.ptmp.tile([128, W0h], F32, tag="g")
            nc.scalar.activation(g[:], ps[:], AF.Gelu_apprx_tanh,
                                 bias=self.lnp_col("cb0", mc))
            nc.vector.tensor_add(h1h[:, mc * W0h:(mc + 1) * W0h],
                                 xth[:, mc * H + H - W0h: mc * H + H].bitcast(F32),
                                 g[:])
        ln1h = self.pst.tile([128, ND * W0h], BF16, tag="ln1h")
        self.layer_norm(lambda dk: h1h[:, dk * W0h:(dk + 1) * W0h],
                        lambda dk: ln1h[:, dk * W0h:(dk + 1) * W0h], W0h,
                        lambda dk: self.lnp_col("s1", dk),
                        lambda dk: self.lnp_col("b1", dk))
        for dk in range(ND):
            nc.vector.tensor_mul(ln1h[:, dk * W0h:(dk + 1) * W0h],
                                 ln1h[:, dk * W0h:(dk + 1) * W0h],
                                 self.mask[:, H - W0h:])
        # initial stashes
        st0 = self.pst.tile([128, ND * 4], BF16, tag="st0")
        st1 = self.pst.tile([128, ND * W0h], BF16, tag="st1")
        for dk in range(ND):
            nc.vector.tensor_copy(st0[:, dk * 4:(dk + 1) * 4],
                                  ln0h[:, dk * H + H - 4: dk * H + H])
            nc.vector.tensor_copy(st1[:, dk * W0h:(dk + 1) * W0h],
                                  ln1h[:, dk * W0h:(dk + 1) * W0h])

        # ---------------- segments -----------------------------------------
        HB, SEG, WP = c.HB, c.SEG, c.W_PH
        LW = HB + SEG  # lnh chunk stride
        for seg in range(c.NSEG):
            h = self.pbuf.tile([128, ND * SEG], F32R, tag="h")
            lnh = self.pbuf.tile([128, ND * LW], BF16, tag="lnh")
            # phase 1+2 per subtile -> h (x2)
            for st0_ in range(0, SEG, WP):
                xt = self.load_xT(H + seg * SEG + st0_, WP)
                self.read_phase(xt, WP, "ltm")
                self.read_phase(xt, WP, "wm",
                                out_at=lambda mc: h[:, mc * SEG + st0_:
                                                    mc * SEG + st0_ + WP])
            # ln0 over segment in W_PH windows
            for w0 in range(0, SEG, WP):
                self.layer_norm(
                    lambda dk: h[:, dk * SEG + w0: dk * SEG + w0 + WP],
                    lambda dk: lnh[:, dk * LW + HB + w0: dk * LW + HB + w0 + WP],
                    WP,
                    lambda dk: self.lnp_col("s0", dk),
                    lambda dk: self.lnp_col("b0", dk))
            for dk in range(ND):
                nc.vector.tensor_copy(lnh[:, dk * LW + HB - 4: dk * LW + HB],
                                      st0[:, dk * 4:(dk + 1) * 4])
            # conv0
            self.conv_layer(0, lambda dk, off, ww: lnh[:, dk * LW + HB + off:
                                                       dk * LW + HB + off + ww],
                            lambda mc, w0, ww: h[:, mc * SEG + w0:
                                                 mc * SEG + w0 + ww], SEG)
            # stash lnh0 tail, then ln1 overwrites lnh
            st0 = self.pst.tile([128, ND * 4], BF16, tag="st0")
            for dk in range(ND):
                nc.vector.tensor_copy(st0[:, dk * 4:(dk + 1) * 4],
                                      lnh[:, dk * LW + HB + SEG - 4: dk * LW + HB + SEG])
            for w0 in range(0, SEG, WP):
                self.layer_norm(
                    lambda dk: h[:, dk * SEG + w0: dk * SEG + w0 + WP],
                    lambda dk: lnh[:, dk * LW + HB + w0: dk * LW + HB + w0 + WP],
                    WP,
                    lambda dk: self.lnp_col("s1", dk),
                    lambda dk: self.lnp_col("b1", dk))
            for dk in range(ND):
                nc.vector.tensor_copy(lnh[:, dk * LW + HB - W0h: dk * LW + HB],
                                      st1[:, dk * W0h:(dk + 1) * W0h])
            self.conv_layer(1, lambda dk, off, ww: lnh[:, dk * LW + HB + off:
                                                       dk * LW + HB + off + ww],
                            lambda mc, w0, ww: h[:, mc * SEG + w0:
                                                 mc * SEG + w0 + ww], SEG)
            st1 = self.pst.tile([128, ND * W0h], BF16, tag="st1")
            for dk in range(ND):
                nc.vector.tensor_copy(
                    st1[:, dk * W0h:(dk + 1) * W0h],
                    lnh[:, dk * LW + HB + SEG - W0h: dk * LW + HB + SEG])
            # postLN -> out^T staging; DMA natural out; write phases
            for st0_ in range(0, SEG, WP):
                ot = self.pot.tile([128, ND * WP], F32R, tag="ot")
                self.layer_norm(
                    lambda dk: h[:, dk * SEG + st0_: dk * SEG + st0_ + WP],
                    lambda dk: ot[:, dk * WP:(dk + 1) * WP], WP,
                    lambda dk: self.lnp_col("ps", dk),
                    lambda dk: self.lnp_col("pb", dk))
                for blk0 in range(0, WP, 128):
                    bw = min(128, WP - blk0)
                    nat = self.pnat.tile([128, c.D], F32, tag="nat")
                    for dk in range(ND):
                        tp = self.ps_tr.tile([128, 128], F32R, tag="tr")
                        nc.tensor.transpose(tp[:bw, :128],
                                            ot[:, dk * WP + blk0: dk * WP + blk0 + bw],
                                            self.ident[:128, :128])
                        nc.scalar.copy(nat[:bw, dk * 128:(dk + 1) * 128],
                                       tp[:bw, :128].bitcast(F32))
                    r0 = seg * SEG + st0_ + blk0
                    nc.sync.dma_start(self.o["out_sh"][r0:r0 + bw, :], nat[:bw, :])
                self.write_phase(ot, WP, acc_wm, acc_ltm)

        nc.sync.dma_start(self.o["wm_stats"], acc_wm[:, 0:c.DC + 1])
        nc.sync.dma_start(self.o["ltm_stats"], acc_ltm[:, 0:c.DC + 1])


# ---------------------------------------------------------------------------
# host side
# ---------------------------------------------------------------------------

def pack_conv_w(w: np.ndarray, cfg: Cfg) -> np.ndarray:
    """[TAPS, D, D] -> [ND, 128, TAPS*ND*128] lhsT blocks (pre-rounded).

    Device tap index means "input at position t - tap*dil", which is
    conv weight w[TAPS-1-tap] under jax's left-padded dilated conv."""
    T, D, _ = w.shape
    nd = cfg.ND
    w = w[::-1]
    # arr[mc, p, tap, dk, q] = w_rev[tap, dk*128+p, mc*128+q]
    a = w.reshape(T, nd, 128, nd, 128)          # tap, dk, p, mc, q
    a = a.transpose(3, 2, 0, 1, 4)              # mc, p, tap, dk, q
    import ml_dtypes
    return np.ascontiguousarray(a.reshape(nd, 128, T * nd * 128)).astype(
        ml_dtypes.bfloat16)


def pack_feat(vecs, cfg: Cfg) -> np.ndarray:
    """list of [D] vectors -> [128, len*ND] (chunk-major columns)."""
    cols = [v.reshape(cfg.ND, 128).T for v in vecs]      # [128, ND] each
    return np.ascontiguousarray(np.concatenate(cols, axis=1).astype(np.float32))


def make_core_inputs(x, cache, wm, params, cfg: Cfg):
    """Build per-core in_maps for the SPMD kernel."""
    c = cfg
    B = x.shape[0]
    S = x.shape[1]
    half = S // 2
    assert half == c.S_CORE
    p = {k: np.asarray(v, dtype=np.float32) for k, v in params.items()}
    shared = {
        "wq_ltm": round_f32r(p["Wq_ltm"]), "wo_ltm": round_f32r(p["Wo_ltm"]),
        "wgr_ltm": round_f32r(np.concatenate(
            [p["wgr_ltm"], np.zeros_like(p["wgr_ltm"])], axis=1)),
        "wq_wm": round_f32r(p["Wq_wm"]), "wo_wm": round_f32r(p["Wo_wm"]),
        "wgr_wm": round_f32r(np.concatenate(
            [p["wgr_wm"], np.zeros_like(p["wgr_wm"])], axis=1)),
        "wv_wm": round_f32r(p["Wv_wm"]), "ws_wm": round_f32r(p["Ws_wm"]),
        "wv_ltm": round_f32r(p["Wv_ltm"]),
        "wgw_pair": round_f32r(np.concatenate([p["wgw_wm"], p["wgw_ltm"]], axis=1)),
        "cw0": pack_conv_w(p["conv_w0"], c),
        "cw1": pack_conv_w(p["conv_w1"], c),
        "lnp": pack_feat([p["ln_s0"], p["ln_b0"], p["ln_s1"], p["ln_b1"],
                          p["post_s"], p["post_b"], p["conv_b0"], p["conv_b1"]], c),
    }
    in_maps = []
    for core in range(2 * B):
        b, hf = core // 2, core % 2
        m = dict(shared)
        xs = np.zeros((c.HALO + c.S_CORE, c.D), np.float32)
        t0 = hf * half
        xs[c.HALO:] = x[b, t0:t0 + half]
        if hf == 1:
            xs[:c.HALO] = x[b, t0 - c.HALO:t0]
            m["mask"] = np.ones((128, c.HALO), np.float32)
        else:
            m["mask"] = np.zeros((128, c.HALO), np.float32)
        xs = round_f32r(xs)
        cb = np.asarray(cache[b], np.float32)
        wmb = np.asarray(wm[b], np.float32)
        cont = wmb[:, :c.DC]
        m["x_sh"] = xs
        m["cache_n"] = round_f32r(cb)
        m["cache_t"] = round_f32r(cb.T)
        m["cont_n"] = round_f32r(cont)
        m["cont_t"] = round_f32r(cont.T)
        m["valid_b"] = np.ascontiguousarray(
            np.broadcast_to(wmb[:, c.DC], (128, c.KW)), np.float32)
        in_maps.append(m)
    return in_maps


def finalize(x, cache, wm, results, cfg: Cfg):
    """Gather per-core outputs; host-side reduce + cache/wm blend."""
    c = cfg
    B = np.asarray(cache).shape[0]
    S = 2 * c.S_CORE
    out = np.empty((B, S, c.D), np.float32)
    upd_cache = np.empty((B, c.K, c.DC), np.float32)
    upd_wm = np.empty((B, c.KW, c.DC + 1), np.float32)
    for b in range(B):
        r0, r1 = results[2 * b], results[2 * b + 1]
        out[b, :c.S_CORE] = r0["out_sh"]
        out[b, c.S_CORE:] = r1["out_sh"]
        ltm = r0["ltm_stats"].astype(np.float64) + r1["ltm_stats"].astype(np.float64)
        wms = r0["wm_stats"].astype(np.float64) + r1["wm_stats"].astype(np.float64)
        cb = np.asarray(cache[b], np.float32)
        wmb = np.asarray(wm[b], np.float32)
        numl, denl = ltm[:, :c.DC], ltm[:, c.DC]
        avgl = (numl / (denl[:, None] + 1e-6)).astype(np.float32)
        al = np.tanh(denl).astype(np.float32)[:, None]
        upd_cache[b] = (1 - al) * cb + al * avgl
        num, den = wms[:, :c.DC], wms[:, c.DC]
        avg = (num / (den[:, None] + 1e-6)).astype(np.float32)
        alpha = np.tanh(den).astype(np.float32)
        cont, valid = wmb[:, :c.DC], wmb[:, c.DC]
        new_cont = (1 - alpha[:, None]) * cont + alpha[:, None] * avg
        new_valid = valid + alpha * (1 - valid)
        upd_wm[b] = np.concatenate([new_cont, new_valid[:, None]], axis=1)
    return out, upd_cache, upd_wm


_PROGRAM_CACHE = {}


def run_cfg(x, cache, wm, params, cfg: Cfg, trace=False):
    if cfg not in _PROGRAM_CACHE:
        _PROGRAM_CACHE[cfg] = K(cfg).build()
    nc = _PROGRAM_CACHE[cfg]
    in_maps = make_core_inputs(x, cache, wm, params, cfg)
    n = len(in_maps)
    res = run_bass_kernel_spmd(nc, in_maps, list(range(n)), trace=trace)
    outs = finalize(x, cache, wm, res.results, cfg)
    return outs, res


def kernel(x, cache, wm, params):
    x = np.asarray(x, np.float32)
    cache = np.asarray(cache, np.float32)
    wm = np.asarray(wm, np.float32)
    outs, _ = run_cfg(x, cache, wm, params, FULL)
    return outs


# revision 28
# speedup vs baseline: 1.0141x; 1.0141x over previous
"""Trainium2 Bass kernel for nn_DecoderCacheLayer (LTM/WM read -> causal dilated
conv stack -> WM/LTM write).

Sharding: 8 cores = (batch b in 0..4) x (sequence half in 0..2). Each core runs
the full fused pipeline on its 2048 tokens (odd halves recompute a 16-token
halo; even halves use the zero-padding semantics via a mask). Cache/WM write
phases produce per-core partial sums ([K,DC+1]/[KW,DC+1]); the host reduces the
two halves of each batch and applies the tiny tanh/blend finalization.

On-device layout is "transposed": features on partitions, tokens on the free
dim. All matmuls run in fp32r (fp32 with 11-bit mantissa, full PE rate).
"""

import numpy as np
from contextlib import ExitStack
from dataclasses import dataclass

import concourse.bass as bass
import concourse.tile as tile
from concourse import bacc, mybir
from concourse.bass_utils import run_bass_kernel_spmd
from concourse.masks import make_identity

F32 = mybir.dt.float32
F32R = mybir.dt.float32r
BF16 = mybir.dt.bfloat16
I32 = mybir.dt.int32
AF = mybir.ActivationFunctionType
ALU = mybir.AluOpType


@dataclass(frozen=True)
class Cfg:
    D: int = 1024          # model dim
    DC: int = 256          # cache dim
    K: int = 64            # LTM slots
    KW: int = 8            # WM slots
    TAPS: int = 5          # conv kernel
    DILS: tuple = (1, 2)   # conv dilations
    S_CORE: int = 2048     # real tokens per core
    HALO: int = 16         # halo tokens recomputed at core start
    SEG: int = 512         # segment length (conv/LN phase width)
    W_PH: int = 256        # subtile width for phase12 / postLN / write phases
    EPS: float = 1e-5
    SC: float = 1.0 / 16.0  # 1/sqrt(DC)

    @property
    def ND(self):
        return self.D // 128

    @property
    def NDC(self):
        return self.DC // 128

    @property
    def HB(self):
        return max((self.TAPS - 1) * d for d in self.DILS)  # lnh lead halo cols

    @property
    def NSEG(self):
        return self.S_CORE // self.SEG


FULL = Cfg()
N_CORES = 8


def round_f32r(x: np.ndarray) -> np.ndarray:
    u = np.ascontiguousarray(x, dtype=np.float32).view(np.uint32)
    r = u + 0x7FF + ((u >> 12) & 1)
    r &= np.uint32(0xFFFFF000)
    return r.view(np.float32)


# ---------------------------------------------------------------------------
# device program
# ---------------------------------------------------------------------------

class K:
    """Builder for the per-core SPMD program."""

    def __init__(self, cfg: Cfg):
        self.cfg = cfg
        nc = bacc.Bacc("TRN2", target_bir_lowering=False, debug=False)
        self.nc = nc
        c = cfg
        dt_in = {}

        def din(name, shape, dt=F32R):
            dt_in[name] = nc.dram_tensor(name, shape, dt, kind="ExternalInput").ap()

        din("x_sh", [c.HALO + c.S_CORE, c.D])  # F32R: host pre-rounds x
        din("cache_n", [c.K, c.DC])
        din("cache_t", [c.DC, c.K])
        din("cont_n", [c.KW, c.DC])
        din("cont_t", [c.DC, c.KW])
        din("valid_b", [128, c.KW], F32)
        din("mask", [128, c.HALO], F32)
        # packed per-feature vectors: s0,b0,s1,b1,post_s,post_b,cb0,cb1
        din("lnp", [128, 8 * c.ND], F32)
        din("wq_ltm", [c.D, c.DC])
        din("wo_ltm", [c.DC, c.D])
        din("wgr_ltm", [c.D, 2])
        din("wq_wm", [c.D, c.DC])
        din("wo_wm", [c.DC, c.D])
        din("wgr_wm", [c.D, 2])
        din("wv_wm", [c.D, c.DC])
        din("ws_wm", [c.D, c.KW])
        din("wgw_pair", [c.D, 2])
        din("wv_ltm", [c.D, c.DC])
        din("cw0", [c.ND, 128, c.TAPS * c.ND * 128], BF16)
        din("cw1", [c.ND, 128, c.TAPS * c.ND * 128], BF16)
        self.i = dt_in
        self.o = {
            "out_sh": nc.dram_tensor("out_sh", [c.S_CORE, c.D], F32,
                                     kind="ExternalOutput").ap(),
            "wm_stats": nc.dram_tensor("wm_stats", [c.KW, c.DC + 1], F32,
                                       kind="ExternalOutput").ap(),
            "ltm_stats": nc.dram_tensor("ltm_stats", [c.K, c.DC + 1], F32,
                                        kind="ExternalOutput").ap(),
        }

    # -- small helpers ------------------------------------------------------

    def mm(self, out, lhsT, rhs, start, stop):
        self.nc.tensor.matmul(out, lhsT, rhs, start=start, stop=stop)

    def build(self):
        cfg = self.cfg
        nc = self.nc
        with tile.TileContext(nc) as tc:
            with ExitStack() as ctx:
                self.tc = tc
                p = lambda name, bufs, **kw: ctx.enter_context(
                    tc.tile_pool(name=name, bufs=bufs, **kw))
                # SBUF pools
                self.pw = p("weights", 1)       # persistent weights / consts
                self.pcw = p("convw", 2)        # streamed conv weights
                self.pbuf = p("bufs", 2)        # h / lnh segment buffers
                self.pxt = p("xt", 2)           # xT staging (also x1/x2 in place)
                self.pot = p("outT", 2)         # postLN ^T staging
                self.pnat = p("nat", 2)         # natural-layout staging tiles
                self.pmid = p("mid", 2)         # qt/rt/vt phase intermediates
                self.ptmp = p("tmp", 2)         # elementwise temporaries
                self.psml = p("small", 2)       # small per-block vectors
                self.pstat = p("stat", 1)       # [1,W] stat vectors
                self.pst = p("stash", 2)        # lnh halo stashes between segments
                self.pacc = p("acc", 1)         # write-phase accumulators
                # PSUM pools (8 banks total)
                self.ps_mm = p("ps_mm", 2, space="PSUM")   # big matmul outputs
                self.ps_st = p("ps_st", 1, space="PSUM")   # [1,2W] LN sums
                self.ps_tr = p("ps_tr", 3, space="PSUM")   # transposes / scores
                self.ps_acc = p("ps_acc", 1, space="PSUM")  # [K,DC+1] write sums
                self._build_body()
        nc.compile()
        return nc

    # -- static tiles -------------------------------------------------------

    def _consts(self):
        nc, c = self.nc, self.cfg
        idf = self.ptmp.tile([128, 128], F32, tag="g")
        make_identity(nc, idf[:])
        self.ident = self.pw.tile([128, 128], F32R, tag="ident")
        nc.vector.tensor_copy(self.ident[:], idf[:])
        onf = self.ptmp.tile([128, 1], F32, tag="tm")
        nc.vector.memset(onf[:], 1.0)
        self.ones_col = self.pw.tile([128, 1], F32R, tag="ones_col")
        nc.vector.tensor_copy(self.ones_col[:], onf[:])
        onr = self.ptmp.tile([1, 128], F32, tag="tm")
        nc.vector.memset(onr[:], 1.0)
        self.ones_row = self.pw.tile([1, 128], F32R, tag="ones_row")
        nc.vector.tensor_copy(self.ones_row[:], onr[:])
        self.eps_t = self.pw.tile([1, 1], F32, tag="eps")
        nc.vector.memset(self.eps_t[:], c.EPS)
        a2f = self.ptmp.tile([128, 2], F32, tag="tm")
        nc.vector.memset(a2f[:, 0:1], 1.0)
        nc.vector.memset(a2f[:, 1:2], 0.0)
        self.aug2 = self.pw.tile([128, 2], F32R, tag="aug2")
        nc.vector.tensor_copy(self.aug2[:], a2f[:])
        # rsqrt Newton constants: magic seed + shift amount, int32 rows
        self.magic_i = self.pw.tile([1, 256], I32, tag="magic")
        nc.vector.memset(self.magic_i[:], 0x5F3759DF)
        self.one_i = self.pw.tile([1, 256], I32, tag="onei")
        nc.vector.memset(self.one_i[:], 1)

    def _load_weights(self):
        nc, c = self.nc, self.cfg
        ND, NDC = c.ND, c.NDC

        # [D, X] weights -> SBUF [128, ND*X] (chunk-major free layout)
        def ld_dx(name, X):
            t = self.pw.tile([128, ND * X], F32R, tag=name)
            nc.sync.dma_start(
                t[:].rearrange("p (n x) -> p n x", n=ND),
                self.i[name].rearrange("(n p) x -> p n x", p=128))
            return t

        # [DC, X] weights -> SBUF [128, NDC*X]
        def ld_cx(name, X):
            t = self.pw.tile([128, NDC * X], F32R, tag=name)
            nc.sync.dma_start(
                t[:].rearrange("p (n x) -> p n x", n=NDC),
                self.i[name].rearrange("(n p) x -> p n x", p=128))
            return t

        self.wq_ltm = ld_dx("wq_ltm", c.DC)
        self.wq_wm = ld_dx("wq_wm", c.DC)
        self.wv_wm = ld_dx("wv_wm", c.DC)
        self.wv_ltm = ld_dx("wv_ltm", c.DC)
        self.ws_wm = ld_dx("ws_wm", c.KW)
        self.wgr_ltm = ld_dx("wgr_ltm", 2)
        self.wgr_wm = ld_dx("wgr_wm", 2)
        self.wgw_pair = ld_dx("wgw_pair", 2)

        self.wo_ltm = ld_cx("wo_ltm", c.D)
        self.wo_wm = ld_cx("wo_wm", c.D)

        self.cache_n = self.pw.tile([c.K, c.DC], F32R, tag="cache_n")
        nc.sync.dma_start(self.cache_n[:], self.i["cache_n"])
        self.cache_t = self.pw.tile([128, NDC * c.K], F32R, tag="cache_t")
        nc.sync.dma_start(
            self.cache_t[:].rearrange("p (n x) -> p n x", n=NDC),
            self.i["cache_t"].rearrange("(n p) x -> p n x", p=128))
        self.cont_n = self.pw.tile([c.KW, c.DC], F32R, tag="cont_n")
        nc.sync.dma_start(self.cont_n[:], self.i["cont_n"])
        self.cont_t = self.pw.tile([128, NDC * c.KW], F32R, tag="cont_t")
        nc.sync.dma_start(
            self.cont_t[:].rearrange("p (n x) -> p n x", n=NDC),
            self.i["cont_t"].rearrange("(n p) x -> p n x", p=128))
        self.valid_b = self.pw.tile([128, c.KW], F32, tag="valid_b")
        nc.sync.dma_start(self.valid_b[:], self.i["valid_b"])
        self.mask = self.pw.tile([128, c.HALO], F32, tag="mask")
        nc.sync.dma_start(self.mask[:], self.i["mask"])
        self.lnp = self.pw.tile([128, 8 * ND], F32, tag="lnp")
        nc.sync.dma_start(self.lnp[:], self.i["lnp"])

    def lnp_col(self, which, dk):
        # order: s0,b0,s1,b1,post_s,post_b,cb0,cb1
        idx = {"s0": 0, "b0": 1, "s1": 2, "b1": 3,
               "ps": 4, "pb": 5, "cb0": 6, "cb1": 7}[which]
        return self.lnp[:, idx * self.cfg.ND + dk: idx * self.cfg.ND + dk + 1]

    # -- building blocks ----------------------------------------------------

    def load_xT(self, row0, W):
        """DMA x rows [row0, row0+W) and transpose into an f32r ^T tile
        [128, ND*W] (feature chunk-major)."""
        nc, c = self.nc, self.cfg
        xt = self.pxt.tile([128, c.ND * W], F32R, tag="xt")
        for blk0 in range(0, W, 128):
            bw = min(128, W - blk0)
            nat = self.pnat.tile([128, c.D], F32R, tag="nat")
            nc.sync.dma_start(nat[:bw, :], self.i["x_sh"][row0 + blk0:
                                                          row0 + blk0 + bw, :])
            for dk in range(c.ND):
                tp = self.ps_tr.tile([128, 128], F32R, tag="tr")
                nc.tensor.transpose(tp[:128, :bw],
                                    nat[:bw, dk * 128:(dk + 1) * 128],
                                    self.ident[:bw, :bw])
                nc.scalar.copy(xt[:, dk * W + blk0: dk * W + blk0 + bw],
                               tp[:128, :bw])
        return xt

    def read_phase(self, xt, W, kind, out_at=None):
        """LTM/WM read: xt (in/out, f32r ^T [128, ND*W]) updated in place:
        x <- x + sigmoid(x@wgr) * ((softmax(x@Wq @ memT)*[valid]) @ mem @ Wo)."""
        nc, c = self.nc, self.cfg
        ND, NDC = c.ND, c.NDC
        if kind == "ltm":
            wq, wo, wgr = self.wq_ltm, self.wo_ltm, self.wgr_ltm
            mem_n, mem_t, nk = self.cache_n, self.cache_t, c.K
        else:
            wq, wo, wgr = self.wq_wm, self.wo_wm, self.wgr_wm
            mem_n, mem_t, nk = self.cont_n, self.cont_t, c.KW

        # q^T = (x @ Wq)^T   [DC, W]
        qt = self.pmid.tile([128, NDC * W], F32R, tag="qt")
        for dcc in range(NDC):
            ps = self.ps_mm.tile([128, W], F32, tag="mm")
            for dk in range(ND):
                self.mm(ps[:], wq[:, dk * c.DC + dcc * 128: dk * c.DC + dcc * 128 + 128],
                        xt[:, dk * W:(dk + 1) * W], dk == 0, dk == ND - 1)
            nc.scalar.copy(qt[:, dcc * W:(dcc + 1) * W], ps[:])
        # attention per 128-token block; the sigmoid read-gate is folded into
        # the attention weights (linear in the read), so the residual is a
        # plain add.
        at = self.ptmp.tile([nk, W], F32R, tag="at")
        for blk0 in range(0, W, 128):
            bw = min(128, W - blk0)
            # natural gate logits [bw, 2] (col 0 = wgr, col 1 = zero pad)
            psg = self.ps_tr.tile([128, 2], F32, tag="tr")
            for dk in range(ND):
                self.mm(psg[:bw, :], xt[:, dk * W + blk0: dk * W + blk0 + bw],
                        wgr[:, dk * 2:(dk + 1) * 2], dk == 0, dk == ND - 1)
            gn = self.psml.tile([128, 2], F32, tag="gn")
            nc.scalar.activation(gn[:bw, :], psg[:bw, :], AF.Tanh, scale=0.5)
            pss = self.ps_tr.tile([128, nk], F32, tag="tr")
            for dcc in range(NDC):
                self.mm(pss[:bw, :], qt[:, dcc * W + blk0: dcc * W + blk0 + bw],
                        mem_t[:, dcc * nk:(dcc + 1) * nk], dcc == 0, dcc == NDC - 1)
            ex = self.psml.tile([128, nk], F32, tag="ex")
            nc.scalar.activation(ex[:bw, :], pss[:bw, :], AF.Exp, scale=c.SC)
            rs = self.psml.tile([128, 1], F32, tag="rs")
            nc.vector.tensor_reduce(rs[:bw, :], ex[:bw, :], mybir.AxisListType.X,
                                    ALU.add)
            rc = self.psml.tile([128, 1], F32, tag="rc")
            nc.vector.reciprocal(rc[:bw, :], rs[:bw, :])
            # rc *= sigmoid(gate) = 0.5*tanh(0.5 g) + 0.5
            sig = self.psml.tile([128, 1], F32, tag="sig")
            nc.vector.tensor_scalar(sig[:bw, :], gn[:bw, 0:1], 0.5, 0.5,
                                    ALU.mult, ALU.add)
            nc.vector.tensor_mul(rc[:bw, :], rc[:bw, :], sig[:bw, :])
            an = self.psml.tile([128, nk], F32R, tag="an")
            if kind == "wm":
                nc.vector.scalar_tensor_tensor(an[:bw, :], ex[:bw, :], rc[:bw, :],
                                               self.valid_b[:bw, :],
                                               ALU.mult, ALU.mult)
            else:
                nc.vector.tensor_scalar_mul(an[:bw, :], ex[:bw, :], rc[:bw, :])
            ptt = self.ps_tr.tile([128, 128], F32R, tag="tr")
            nc.tensor.transpose(ptt[:nk, :bw], an[:bw, :nk], self.ident[:bw, :bw])
            nc.vector.tensor_copy(at[:, blk0:blk0 + bw], ptt[:nk, :bw])
        # read^T [DC, W] then out-proj + residual, all per chunk
        rt = self.pmid.tile([128, NDC * W], F32R, tag="rt")
        for dcc in range(NDC):
            ps = self.ps_mm.tile([128, W], F32, tag="mm")
            self.mm(ps[:], mem_n[:, dcc * 128:(dcc + 1) * 128], at[:], True, True)
            nc.scalar.copy(rt[:, dcc * W:(dcc + 1) * W], ps[:])
        for mc in range(ND):
            ps = self.ps_mm.tile([128, W], F32, tag="mm")
            for dcc in range(NDC):
                self.mm(ps[:], wo[:, dcc * c.D + mc * 128: dcc * c.D + mc * 128 + 128],
                        rt[:, dcc * W:(dcc + 1) * W], dcc == 0, dcc == NDC - 1)
            dst = out_at(mc) if out_at is not None else xt[:, mc * W:(mc + 1) * W]
            nc.vector.tensor_add(dst,
                                 xt[:, mc * W:(mc + 1) * W].bitcast(F32), ps[:])

    def layer_norm(self, in_at, out_at, W, s_col, b_col):
        """Partition-axis LN in ^T layout. in_at/out_at: fn(dk)->AP [128, W].
        s_col/b_col: fn(dk)->AP [128,1]."""
        nc, c = self.nc, self.cfg
        ND = c.ND
        psst = self.ps_st.tile([1, 2 * W], F32, tag="st")
        pssum = psst[:, 0:W]
        pssq = psst[:, W:2 * W]
        for dk in range(ND):
            self.mm(pssum, self.ones_col[:], in_at(dk), dk == 0, dk == ND - 1)
        for dk in range(ND):
            a = in_at(dk)
            sq = self.ptmp.tile([128, W], F32R, tag="lntmp")
            nc.vector.tensor_mul(sq[:], a.bitcast(F32), a.bitcast(F32))
            self.mm(pssq, self.ones_col[:], sq[:], dk == 0, dk == ND - 1)
        inv_d = 1.0 / c.D
        meanf = self.pstat.tile([1, W], F32, tag="meanf")
        nc.scalar.mul(meanf[:], pssum, inv_d)
        msq = self.pstat.tile([1, W], F32, tag="msq")
        nc.scalar.square(msq[:], meanf[:])
        var = self.pstat.tile([1, W], F32, tag="var")
        nc.vector.scalar_tensor_tensor(var[:], pssq, inv_d, msq[:],
                                       ALU.mult, ALU.subtract)
        # rstd = exp(-0.5 * ln(var + eps))
        lnv = self.pstat.tile([1, W], F32, tag="lnv")
        nc.scalar.activation(lnv[:], var[:], AF.Ln, bias=self.eps_t[:])
        rstdf = self.pstat.tile([1, W], F32, tag="rstdf")
        nc.scalar.activation(rstdf[:], lnv[:], AF.Exp, scale=-0.5)
        rstd = self.pstat.tile([1, W], F32R, tag="rstd")
        nc.vector.tensor_copy(rstd[:], rstdf[:])
        mean = self.pstat.tile([1, W], F32R, tag="mean")
        nc.vector.tensor_copy(mean[:], meanf[:])
        psmb = self.ps_mm.tile([128, W], F32, tag="mm")
        self.mm(psmb[:], self.ones_row[:], mean[:], True, True)
        psrb = self.ps_mm.tile([128, W], F32, tag="mm")
        self.mm(psrb[:], self.ones_row[:], rstd[:], True, True)
        mb = self.pstat.tile([128, W], F32, tag="meanb")
        nc.scalar.copy(mb[:], psmb[:])
        rb = self.pstat.tile([128, W], F32, tag="rstdb")
        nc.scalar.copy(rb[:], psrb[:])
        for dk in range(ND):
            t1 = self.ptmp.tile([128, W], F32, tag="lntmp")
            nc.vector.tensor_sub(t1[:], in_at(dk).bitcast(F32), mb[:])
            dst = out_at(dk)
            dstr = dst.bitcast(F32) if dst.dtype == F32R else dst
            nc.vector.scalar_tensor_tensor(dst, t1[:], s_col(dk), rb[:],
                                           ALU.mult, ALU.mult)
            nc.vector.tensor_scalar_add(dst, dstr, b_col(dk))

    def conv_layer(self, layer, lnh_at, h_at, W, halo_cb=None):
        """One causal dilated conv + gelu + residual (bf16 weights/inputs).
        lnh_at(dk, off, ww) -> AP window; h_at(mc, w0, ww) -> dest cols."""
        nc, c = self.nc, self.cfg
        dil = c.DILS[layer]
        cw = self.i[f"cw{layer}"]
        bias = lambda mc: self.lnp_col(f"cb{layer}", mc)
        for mc in range(c.ND):
            wt = self.pcw.tile([128, c.TAPS * c.ND * 128], BF16, tag="cw")
            nc.sync.dma_start(wt[:], cw[mc])
            if halo_cb is not None:
                halo_cb(mc, wt)
            for w0 in range(0, W, 512):
                ww = min(512, W - w0)
                ps = self.ps_mm.tile([128, ww], F32, tag="mm")
                n = 0
                for tap in range(c.TAPS):
                    for dk in range(c.ND):
                        self.mm(ps[:],
                                wt[:, (tap * c.ND + dk) * 128:(tap * c.ND + dk + 1) * 128],
                                lnh_at(dk, w0 - tap * dil, ww),
                                n == 0, n == c.TAPS * c.ND - 1)
                        n += 1
                g = self.ptmp.tile([128, ww], F32, tag="g")
                nc.scalar.activation(g[:], ps[:], AF.Gelu_apprx_tanh, bias=bias(mc))
                dst = h_at(mc, w0, ww)
                nc.vector.tensor_add(dst, dst.bitcast(F32), g[:])

    def write_phase(self, ot, W, acc_wm, acc_ltm):
        """WM/LTM write partial sums from out^T tile ot [128, ND*W] (f32r)."""
        nc, c = self.nc, self.cfg
        ND, NDC = c.ND, c.NDC

        # v^T projections [DC, W] for wm and ltm
        def vproj(wv, tag):
            vt = self.pmid.tile([128, NDC * W], F32R, tag=tag)
            for dcc in range(NDC):
                ps = self.ps_mm.tile([128, W], F32, tag="mm")
                for dk in range(ND):
                    self.mm(ps[:],
                            wv[:, dk * c.DC + dcc * 128: dk * c.DC + dcc * 128 + 128],
                            ot[:, dk * W:(dk + 1) * W], dk == 0, dk == ND - 1)
                nc.scalar.copy(vt[:, dcc * W:(dcc + 1) * W], ps[:])
            return vt

        vwt = vproj(self.wv_wm, "vwt")
        vlt = vproj(self.wv_ltm, "vlt")

        ps_nw = self.ps_acc.tile([c.KW, c.DC + 2], F32, tag="accw")
        ps_nl = self.ps_acc.tile([c.K, c.DC + 2], F32, tag="accl")
        nblk = (W + 127) // 128
        for bi in range(nblk):
            blk0 = bi * 128
            bw = min(128, W - blk0)
            # natural gates [bw, 2] = sigmoid(out @ [wgw_wm | wgw_ltm])
            psg = self.ps_tr.tile([128, 2], F32, tag="tr")
            for dk in range(ND):
                self.mm(psg[:bw, :], ot[:, dk * W + blk0: dk * W + blk0 + bw],
                        self.wgw_pair[:, dk * 2:(dk + 1) * 2], dk == 0, dk == ND - 1)
            gnat = self.psml.tile([128, 2], F32, tag="gnat")
            nc.scalar.activation(gnat[:bw, :], psg[:bw, :], AF.Tanh, scale=0.5)
            nc.vector.tensor_scalar(gnat[:bw, :], gnat[:bw, :], 0.5, 0.5,
                                    ALU.mult, ALU.add)
            # ---- WM: softmax(out@Ws) * gate, natural layout
            psw = self.ps_tr.tile([128, c.KW], F32, tag="tr")
            for dk in range(ND):
                self.mm(psw[:bw, :], ot[:, dk * W + blk0: dk * W + blk0 + bw],
                        self.ws_wm[:, dk * c.KW:(dk + 1) * c.KW],
                        dk == 0, dk == ND - 1)
            exw = self.psml.tile([128, c.KW], F32, tag="ex")
            nc.scalar.activation(exw[:bw, :], psw[:bw, :], AF.Exp)
            rsw = self.psml.tile([128, 1], F32, tag="rs")
            nc.vector.tensor_reduce(rsw[:bw, :], exw[:bw, :], mybir.AxisListType.X,
                                    ALU.add)
            rgw = self.psml.tile([128, 1], F32, tag="rg")
            nc.vector.reciprocal(rgw[:bw, :], rsw[:bw, :])
            nc.vector.tensor_mul(rgw[:bw, :], rgw[:bw, :], gnat[:bw, 0:1])
            wv = self.psml.tile([128, c.KW], F32R, tag="wv")
            nc.vector.tensor_scalar_mul(wv[:bw, :], exw[:bw, :], rgw[:bw, :])
            # augmented natural v [bw, DC+2]
            vwa = self.ptmp.tile([128, c.DC + 2], F32R, tag="vaug")
            for dcc in range(NDC):
                pvt = self.ps_tr.tile([128, 128], F32R, tag="tr")
                nc.tensor.transpose(pvt[:bw, :128],
                                    vwt[:, dcc * W + blk0: dcc * W + blk0 + bw],
                                    self.ident[:128, :128])
                nc.vector.tensor_copy(vwa[:bw, dcc * 128:(dcc + 1) * 128],
                                      pvt[:bw, :128])
            nc.vector.tensor_copy(vwa[:bw, c.DC:c.DC + 2],
                                  self.aug2[:bw, :])
            self.mm(ps_nw[:], wv[:bw, :], vwa[:bw, :], bi == 0, bi == nblk - 1)
            # ---- LTM: softmax(v@cacheT * sc) * gate
            psl = self.ps_tr.tile([128, c.K], F32, tag="tr")
            for dcc in range(NDC):
                self.mm(psl[:bw, :], vlt[:, dcc * W + blk0: dcc * W + blk0 + bw],
                        self.cache_t[:, dcc * c.K:(dcc + 1) * c.K],
                        dcc == 0, dcc == NDC - 1)
            exl = self.psml.tile([128, c.K], F32, tag="exl")
            nc.scalar.activation(exl[:bw, :], psl[:bw, :], AF.Exp, scale=c.SC)
            rsl = self.psml.tile([128, 1], F32, tag="rs")
            nc.vector.tensor_reduce(rsl[:bw, :], exl[:bw, :], mybir.AxisListType.X,
                                    ALU.add)
            rgl = self.psml.tile([128, 1], F32, tag="rg")
            nc.vector.reciprocal(rgl[:bw, :], rsl[:bw, :])
            nc.vector.tensor_mul(rgl[:bw, :], rgl[:bw, :], gnat[:bw, 1:2])
            wl = self.psml.tile([128, c.K], F32R, tag="wl")
            nc.vector.tensor_scalar_mul(wl[:bw, :], exl[:bw, :], rgl[:bw, :])
            vla = self.ptmp.tile([128, c.DC + 2], F32R, tag="vaug")
            for dcc in range(NDC):
                pvt = self.ps_tr.tile([128, 128], F32R, tag="tr")
                nc.tensor.transpose(pvt[:bw, :128],
                                    vlt[:, dcc * W + blk0: dcc * W + blk0 + bw],
                                    self.ident[:128, :128])
                nc.vector.tensor_copy(vla[:bw, dcc * 128:(dcc + 1) * 128],
                                      pvt[:bw, :128])
            nc.vector.tensor_copy(vla[:bw, c.DC:c.DC + 2], self.aug2[:bw, :])
            self.mm(ps_nl[:], wl[:bw, :], vla[:bw, :], bi == 0, bi == nblk - 1)
        nc.vector.tensor_add(acc_wm[:], acc_wm[:], ps_nw[:])
        nc.vector.tensor_add(acc_ltm[:], acc_ltm[:], ps_nl[:])

    # -- main body ----------------------------------------------------------

    def _build_body(self):
        nc, c = self.nc, self.cfg
        ND = c.ND
        self._consts()
        self._load_weights()

        acc_wm = self.pacc.tile([c.KW, c.DC + 2], F32, tag="acc_wm")
        nc.vector.memset(acc_wm[:], 0.0)
        acc_ltm = self.pacc.tile([c.K, c.DC + 2], F32, tag="acc_ltm")
        nc.vector.memset(acc_ltm[:], 0.0)

        # ---------------- prologue: halo tokens [0, HALO) -------------------
        H = c.HALO
        W0h = c.DILS[1] * (c.TAPS - 1)   # h1 halo width (8)
        xth = self.load_xT(0, H)
        self.read_phase(xth, H, "ltm")
        self.read_phase(xth, H, "wm")
        # xth now holds x2 (h0) on halo tokens; ln0 -> masked lnh0_halo
        ln0h = self.pst.tile([128, ND * H], BF16, tag="ln0h")
        self.layer_norm(lambda dk: xth[:, dk * H:(dk + 1) * H],
                        lambda dk: ln0h[:, dk * H:(dk + 1) * H], H,
                        lambda dk: self.lnp_col("s0", dk),
                        lambda dk: self.lnp_col("b0", dk))
        for dk in range(ND):
            nc.vector.tensor_mul(ln0h[:, dk * H:(dk + 1) * H],
                                 ln0h[:, dk * H:(dk + 1) * H],
                                 self.mask[:])
        # halo conv0 (positions [-W0h, 0)) is emitted inside segment 0's
        # conv0 loop so it reuses the same streamed weight tiles.
        h1h = self.pst.tile([128, ND * W0h], F32R, tag="h1h")
        dil0 = c.DILS[0]

        def emit_halo(mc, wt):
            ps = self.ps_mm.tile([128, W0h], F32, tag="mm")
            n = 0
            for tap in range(c.TAPS):
                for dk2 in range(ND):
                    off = dk2 * H + (H - W0h) - tap * dil0
                    self.mm(ps[:],
                            wt[:, (tap * ND + dk2) * 128:(tap * ND + dk2 + 1) * 128],
                            ln0h[:, off: off + W0h], n == 0, n == c.TAPS * ND - 1)
                    n += 1
            g = self.ptmp.tile([128, W0h], F32, tag="g")
            nc.scalar.activation(g[:], ps[:], AF.Gelu_apprx_tanh,
                                 bias=self.lnp_col("cb0", mc))
            nc.vector.tensor_add(h1h[:, mc * W0h:(mc + 1) * W0h],
                                 xth[:, mc * H + H - W0h: mc * H + H].bitcast(F32),
                                 g[:])

        # initial ln0 stash
        st0 = self.pst.tile([128, ND * 4], BF16, tag="st0")
        for dk in range(ND):
            nc.vector.tensor_copy(st0[:, dk * 4:(dk + 1) * 4],
                                  ln0h[:, dk * H + H - 4: dk * H + H])
        st1 = None

        # ---------------- segments -----------------------------------------
        HB, SEG, WP = c.HB, c.SEG, c.W_PH
        LW = HB + SEG  # lnh chunk stride
        for seg in range(c.NSEG):
            h = self.pbuf.tile([128, ND * SEG], F32R, tag="h")
            lnh = self.pbuf.tile([128, ND * LW], BF16, tag="lnh")
            # phase 1+2 per subtile -> h (x2)
            for st0_ in range(0, SEG, WP):
                xt = self.load_xT(H + seg * SEG + st0_, WP)
                self.read_phase(xt, WP, "ltm")
                self.read_phase(xt, WP, "wm",
                                out_at=lambda mc: h[:, mc * SEG + st0_:
                                                    mc * SEG + st0_ + WP])
            # ln0 over segment in W_PH windows
            for w0 in range(0, SEG, WP):
                self.layer_norm(
                    lambda dk: h[:, dk * SEG + w0: dk * SEG + w0 + WP],
                    lambda dk: lnh[:, dk * LW + HB + w0: dk * LW + HB + w0 + WP],
                    WP,
                    lambda dk: self.lnp_col("s0", dk),
                    lambda dk: self.lnp_col("b0", dk))
            for dk in range(ND):
                nc.vector.tensor_copy(lnh[:, dk * LW + HB - 4: dk * LW + HB],
                                      st0[:, dk * 4:(dk + 1) * 4])
            # conv0 (segment 0 also emits the halo mini-conv per mc)
            self.conv_layer(0, lambda dk, off, ww: lnh[:, dk * LW + HB + off:
                                                       dk * LW + HB + off + ww],
                            lambda mc, w0, ww: h[:, mc * SEG + w0:
                                                 mc * SEG + w0 + ww], SEG,
                            halo_cb=emit_halo if seg == 0 else None)
            if seg == 0:
                ln1h = self.pst.tile([128, ND * W0h], BF16, tag="ln1h")
                self.layer_norm(lambda dk: h1h[:, dk * W0h:(dk + 1) * W0h],
                                lambda dk: ln1h[:, dk * W0h:(dk + 1) * W0h], W0h,
                                lambda dk: self.lnp_col("s1", dk),
                                lambda dk: self.lnp_col("b1", dk))
                for dk in range(ND):
                    nc.vector.tensor_mul(ln1h[:, dk * W0h:(dk + 1) * W0h],
                                         ln1h[:, dk * W0h:(dk + 1) * W0h],
                                         self.mask[:, H - W0h:])
                st1 = self.pst.tile([128, ND * W0h], BF16, tag="st1")
                for dk in range(ND):
                    nc.vector.tensor_copy(st1[:, dk * W0h:(dk + 1) * W0h],
                                          ln1h[:, dk * W0h:(dk + 1) * W0h])
            # stash lnh0 tail, then ln1 overwrites lnh
            st0 = self.pst.tile([128, ND * 4], BF16, tag="st0")
            for dk in range(ND):
                nc.vector.tensor_copy(st0[:, dk * 4:(dk + 1) * 4],
                                      lnh[:, dk * LW + HB + SEG - 4: dk * LW + HB + SEG])
            for w0 in range(0, SEG, WP):
                self.layer_norm(
                    lambda dk: h[:, dk * SEG + w0: dk * SEG + w0 + WP],
                    lambda dk: lnh[:, dk * LW + HB + w0: dk * LW + HB + w0 + WP],
                    WP,
                    lambda dk: self.lnp_col("s1", dk),
                    lambda dk: self.lnp_col("b1", dk))
            for dk in range(ND):
                nc.vector.tensor_copy(lnh[:, dk * LW + HB - W0h: dk * LW + HB],
                                      st1[:, dk * W0h:(dk + 1) * W0h])
            self.conv_layer(1, lambda dk, off, ww: lnh[:, dk * LW + HB + off:
                                                       dk * LW + HB + off + ww],
                            lambda mc, w0, ww: h[:, mc * SEG + w0:
                                                 mc * SEG + w0 + ww], SEG)
            st1 = self.pst.tile([128, ND * W0h], BF16, tag="st1")
            for dk in range(ND):
                nc.vector.tensor_copy(
                    st1[:, dk * W0h:(dk + 1) * W0h],
                    lnh[:, dk * LW + HB + SEG - W0h: dk * LW + HB + SEG])
            # postLN -> out^T staging; DMA natural out; write phases
            for st0_ in range(0, SEG, WP):
                ot = self.pot.tile([128, ND * WP], F32R, tag="ot")
                self.layer_norm(
                    lambda dk: h[:, dk * SEG + st0_: dk * SEG + st0_ + WP],
                    lambda dk: ot[:, dk * WP:(dk + 1) * WP], WP,
                    lambda dk: self.lnp_col("ps", dk),
                    lambda dk: self.lnp_col("pb", dk))
                for blk0 in range(0, WP, 128):
                    bw = min(128, WP - blk0)
                    nat = self.pnat.tile([128, c.D], F32, tag="nat")
                    for dk in range(ND):
                        tp = self.ps_tr.tile([128, 128], F32R, tag="tr")
                        nc.tensor.transpose(tp[:bw, :128],
                                            ot[:, dk * WP + blk0: dk * WP + blk0 + bw],
                                            self.ident[:128, :128])
                        nc.scalar.copy(nat[:bw, dk * 128:(dk + 1) * 128],
                                       tp[:bw, :128].bitcast(F32))
                    r0 = seg * SEG + st0_ + blk0
                    nc.sync.dma_start(self.o["out_sh"][r0:r0 + bw, :], nat[:bw, :])
                self.write_phase(ot, WP, acc_wm, acc_ltm)

        nc.sync.dma_start(self.o["wm_stats"], acc_wm[:, 0:c.DC + 1])
        nc.sync.dma_start(self.o["ltm_stats"], acc_ltm[:, 0:c.DC + 1])


# ---------------------------------------------------------------------------
# host side
# ---------------------------------------------------------------------------

def pack_conv_w(w: np.ndarray, cfg: Cfg) -> np.ndarray:
    """[TAPS, D, D] -> [ND, 128, TAPS*ND*128] lhsT blocks (pre-rounded).

    Device tap index means "input at position t - tap*dil", which is
    conv weight w[TAPS-1-tap] under jax's left-padded dilated conv."""
    T, D, _ = w.shape
    nd = cfg.ND
    w = w[::-1]
    # arr[mc, p, tap, dk, q] = w_rev[tap, dk*128+p, mc*128+q]
    a = w.reshape(T, nd, 128, nd, 128)          # tap, dk, p, mc, q
    a = a.transpose(3, 2, 0, 1, 4)              # mc, p, tap, dk, q
    import ml_dtypes
    return np.ascontiguousarray(a.reshape(nd, 128, T * nd * 128)).astype(
        ml_dtypes.bfloat16)


def pack_feat(vecs, cfg: Cfg) -> np.ndarray:
    """list of [D] vectors -> [128, len*ND] (chunk-major columns)."""
    cols = [v.reshape(cfg.ND, 128).T for v in vecs]      # [128, ND] each
    return np.ascontiguousarray(np.concatenate(cols, axis=1).astype(np.float32))


def make_core_inputs(x, cache, wm, params, cfg: Cfg):
    """Build per-core in_maps for the SPMD kernel."""
    c = cfg
    B = x.shape[0]
    S = x.shape[1]
    half = S // 2
    assert half == c.S_CORE
    p = {k: np.asarray(v, dtype=np.float32) for k, v in params.items()}
    shared = {
        "wq_ltm": round_f32r(p["Wq_ltm"]), "wo_ltm": round_f32r(p["Wo_ltm"]),
        "wgr_ltm": round_f32r(np.concatenate(
            [p["wgr_ltm"], np.zeros_like(p["wgr_ltm"])], axis=1)),
        "wq_wm": round_f32r(p["Wq_wm"]), "wo_wm": round_f32r(p["Wo_wm"]),
        "wgr_wm": round_f32r(np.concatenate(
            [p["wgr_wm"], np.zeros_like(p["wgr_wm"])], axis=1)),
        "wv_wm": round_f32r(p["Wv_wm"]), "ws_wm": round_f32r(p["Ws_wm"]),
        "wv_ltm": round_f32r(p["Wv_ltm"]),
        "wgw_pair": round_f32r(np.concatenate([p["wgw_wm"], p["wgw_ltm"]], axis=1)),
        "cw0": pack_conv_w(p["conv_w0"], c),
        "cw1": pack_conv_w(p["conv_w1"], c),
        "lnp": pack_feat([p["ln_s0"], p["ln_b0"], p["ln_s1"], p["ln_b1"],
                          p["post_s"], p["post_b"], p["conv_b0"], p["conv_b1"]], c),
    }
    in_maps = []
    for core in range(2 * B):
        b, hf = core // 2, core % 2
        m = dict(shared)
        xs = np.zeros((c.HALO + c.S_CORE, c.D), np.float32)
        t0 = hf * half
        xs[c.HALO:] = x[b, t0:t0 + half]
        if hf == 1:
            xs[:c.HALO] = x[b, t0 - c.HALO:t0]
            m["mask"] = np.ones((128, c.HALO), np.float32)
        else:
            m["mask"] = np.zeros((128, c.HALO), np.float32)
        xs = round_f32r(xs)
        cb = np.asarray(cache[b], np.float32)
        wmb = np.asarray(wm[b], np.float32)
        cont = wmb[:, :c.DC]
        m["x_sh"] = xs
        m["cache_n"] = round_f32r(cb)
        m["cache_t"] = round_f32r(cb.T)
        m["cont_n"] = round_f32r(cont)
        m["cont_t"] = round_f32r(cont.T)
        m["valid_b"] = np.ascontiguousarray(
            np.broadcast_to(wmb[:, c.DC], (128, c.KW)), np.float32)
        in_maps.append(m)
    return in_maps


def finalize(x, cache, wm, results, cfg: Cfg):
    """Gather per-core outputs; host-side reduce + cache/wm blend."""
    c = cfg
    B = np.asarray(cache).shape[0]
    S = 2 * c.S_CORE
    out = np.empty((B, S, c.D), np.float32)
    upd_cache = np.empty((B, c.K, c.DC), np.float32)
    upd_wm = np.empty((B, c.KW, c.DC + 1), np.float32)
    for b in range(B):
        r0, r1 = results[2 * b], results[2 * b + 1]
        out[b, :c.S_CORE] = r0["out_sh"]
        out[b, c.S_CORE:] = r1["out_sh"]
        ltm = r0["ltm_stats"].astype(np.float64) + r1["ltm_stats"].astype(np.float64)
        wms = r0["wm_stats"].astype(np.float64) + r1["wm_stats"].astype(np.float64)
        cb = np.asarray(cache[b], np.float32)
        wmb = np.asarray(wm[b], np.float32)
        numl, denl = ltm[:, :c.DC], ltm[:, c.DC]
        avgl = (numl / (denl[:, None] + 1e-6)).astype(np.float32)
        al = np.tanh(denl).astype(np.float32)[:, None]
        upd_cache[b] = (1 - al) * cb + al * avgl
        num, den = wms[:, :c.DC], wms[:, c.DC]
        avg = (num / (den[:, None] + 1e-6)).astype(np.float32)
        alpha = np.tanh(den).astype(np.float32)
        cont, valid = wmb[:, :c.DC], wmb[:, c.DC]
        new_cont = (1 - alpha[:, None]) * cont + alpha[:, None] * avg
        new_valid = valid + alpha * (1 - valid)
        upd_wm[b] = np.concatenate([new_cont, new_valid[:, None]], axis=1)
    return out, upd_cache, upd_wm


_PROGRAM_CACHE = {}


def run_cfg(x, cache, wm, params, cfg: Cfg, trace=False):
    if cfg not in _PROGRAM_CACHE:
        _PROGRAM_CACHE[cfg] = K(cfg).build()
    nc = _PROGRAM_CACHE[cfg]
    in_maps = make_core_inputs(x, cache, wm, params, cfg)
    n = len(in_maps)
    res = run_bass_kernel_spmd(nc, in_maps, list(range(n)), trace=trace)
    outs = finalize(x, cache, wm, res.results, cfg)
    return outs, res


def kernel(x, cache, wm, params):
    x = np.asarray(x, np.float32)
    cache = np.asarray(cache, np.float32)
    wm = np.asarray(wm, np.float32)
    outs, _ = run_cfg(x, cache, wm, params, FULL)
    return outs
